# revision 1
# baseline (speedup 1.0000x reference)
"""GNN message-passing kernel (4x GraphConv + BN + ELU, mean-pool, MLP, log_softmax)
for 8 Trainium2 NeuronCores.

Strategy:
  - Shard nodes (and their incident in-edges) across 8 cores by contiguous node
    ranges (batch stays sorted -> graph segments stay local-ish).
  - Aggregation: dma_gather (bf16, small elems) of source-node features from a
    256B-strided replicated table, then one-hot matmul accumulation into PSUM
    per 64-node destination tile.  Host precomputes all index structures.
  - Feature-major (fm) layout [C, nodes] for matmuls/stats; node-major (nm)
    [128, blocks, C] for BN-affine+ELU; PE transposes convert.
  - Collectives: AllReduce for BN stats + pooled logits; AllGather for the
    per-layer node-feature tables.
"""
import inspect
import textwrap

import ml_dtypes
import numpy as np

import concourse.bacc as bacc
import concourse.bass as bass
import concourse.mybir as mybir
import concourse.tile as tile
from concourse.bass_utils import run_bass_kernel_spmd

# ---- relax dma_gather's 256B elem restriction to 32B (ucode supports it; the
# stride stays 256B-quantized which we honor with 128-bf16-elem table rows).
_src = inspect.getsource(bass.BassGpSimd.dma_gather)
_src = _src.replace("elem_size_bytes % 256 == 0", "elem_size_bytes % 32 == 0")
_src = "def dma_gather" + _src.split("def dma_gather", 1)[1]
_ns = dict(bass.__dict__)
exec(textwrap.dedent(_src), _ns)
bass.BassGpSimd.dma_gather = _ns["dma_gather"]

F32 = mybir.dt.float32
BF16 = mybir.dt.bfloat16
I16 = mybir.dt.int16
AF = mybir.ActivationFunctionType
ALU = mybir.AluOpType

N = 100000
E = 3200000
NCORES = 8
NSHARD = N // NCORES          # 12500
NBLK = (NSHARD + 127) // 128  # 98
NPAD = NBLK * 128             # 12544
NUM_GRAPHS = 256
EPS = 1e-5
W = 128                       # dst-tile width / onehot width
NTILES = (NSHARD + W - 1) // W   # 98
# int16 idx used as non-negative offset from a per-bucket base row
BBASE = [0, 32768, 65536, 98304]
NBUCK = 4
GRP = 7                       # tiles per gather group
NGRP = NTILES // GRP          # 14
NODE_CH = 512                 # fm node-chunk for the dense matmuls
TSTRIDE = 128                 # bf16 elems per table row = 256B

# (C_in, C_out, D_agg, project_before)
LAYERS = [(64, 16, 16, True), (16, 32, 16, False), (32, 32, 32, False), (32, 64, 32, False)]


def _preprocess(x, edge_index, batch):
    src = np.asarray(edge_index[0], dtype=np.int64)
    dst = np.asarray(edge_index[1], dtype=np.int64)
    batch = np.asarray(batch, dtype=np.int64)
    x = np.asarray(x, dtype=np.float32)

    core = dst // NSHARD
    dstloc = dst - core * NSHARD
    tl = dstloc // W
    wloc = dstloc - tl * W
    bk = src // 32768

    # per (core, tile, bucket) edge counts -> uniform segment lengths
    key = (core * NTILES + tl) * NBUCK + bk
    cnt = np.bincount(key, minlength=NCORES * NTILES * NBUCK).reshape(NCORES, NTILES, NBUCK)
    seg = cnt.max(axis=0)
    seg = ((seg + 127) // 128) * 128 * (seg > 0)       # [NTILES, NBUCK]

    # chunk layout: for g in groups: for b in buckets: for t in group: seg(t,b)
    seg_ch = seg // 128
    # global column index of each (t,b) block start
    col0 = np.zeros((NTILES, NBUCK), dtype=np.int64)
    gb_cols = np.zeros((NGRP, NBUCK), dtype=np.int64)   # cols per (group,bucket)
    g_col0 = np.zeros(NGRP + 1, dtype=np.int64)
    c = 0
    for g in range(NGRP):
        g_col0[g] = c
        for b in range(NBUCK):
            for t in range(g * GRP, (g + 1) * GRP):
                col0[t, b] = c
                c += seg_ch[t, b]
            gb_cols[g, b] = c - (col0[g * GRP, b])
    g_col0[NGRP] = c
    total_ch = c
    total_e = total_ch * 128

    # per-core edge placement
    order = np.argsort(core * np.int64(NTILES * NBUCK) + tl * NBUCK + bk, kind="stable")
    s_src, s_tl, s_bk, s_w, s_core = src[order], tl[order], bk[order], wloc[order], core[order]

    idx16 = np.zeros((NCORES, total_e), dtype=np.int16)
    dstw = np.full((NCORES, total_e), -1.0, dtype=np.float32)
    pos = 0
    for k in range(NCORES):
        for t in range(NTILES):
            for b in range(NBUCK):
                n = cnt[k, t, b]
                if n:
                    sl = slice(pos, pos + n)
                    base = col0[t, b] * 128
                    p = np.arange(n)
                    idx16[k, base + p] = (s_src[sl] - BBASE[b]).astype(np.int16)
                    dstw[k, base + p] = s_w[sl]
                    pos += n
    assert pos == E

    # wrapped (16) + replicated (x8) idx layout: position i -> [i%16, i//16]
    idx_w = idx16.reshape(NCORES, total_e // 16, 16).transpose(0, 2, 1)  # [NCORES,16,tot/16]
    idx_rep = np.tile(idx_w, (1, 8, 1))                                  # [NCORES,128,tot/16]
    # dstw [core, 128, total_ch]: position i=(c*128+p) -> [p, c]
    dstw_pc = dstw.reshape(NCORES, total_ch, 128).transpose(0, 2, 1).astype(ml_dtypes.bfloat16)

    # x shards, fm layout
    x_fm = np.zeros((NCORES, 64, NPAD), dtype=np.float32)
    for k in range(NCORES):
        x_fm[k, :, :NSHARD] = x[k * NSHARD:(k + 1) * NSHARD].T

    # in-degree reciprocal, fm-tiled [32, NPAD]
    indeg = np.bincount(dst, minlength=N).astype(np.float32)
    recip = 1.0 / np.maximum(indeg, 1.0)
    recip_fm = np.zeros((NCORES, 32, NPAD), dtype=np.float32)
    for k in range(NCORES):
        recip_fm[k, :, :NSHARD] = recip[k * NSHARD:(k + 1) * NSHARD][None, :]

    # graph one-hot [128, NBLK*256] bf16 per core
    gh = np.zeros((NCORES, NPAD, NUM_GRAPHS), dtype=np.float32)
    for k in range(NCORES):
        gi = batch[k * NSHARD:(k + 1) * NSHARD]
        gh[k, np.arange(NSHARD), gi] = 1.0
    gh = gh.reshape(NCORES, NBLK, 128, NUM_GRAPHS).transpose(0, 2, 1, 3)
    gh = gh.reshape(NCORES, 128, NBLK * NUM_GRAPHS).astype(ml_dtypes.bfloat16)

    gcnt = np.bincount(batch, minlength=NUM_GRAPHS).astype(np.float32)
    grecip = (1.0 / np.maximum(gcnt, 1.0)).reshape(2, 128).T.astype(np.float32)  # [128,2]

    meta = dict(seg_ch=seg_ch, col0=col0, gb_cols=gb_cols, g_col0=g_col0,
                total_ch=int(total_ch))
    percore = dict(idx=idx_rep, dstw=dstw_pc, x_fm=x_fm, recip_fm=recip_fm, gh=gh)
    shared = dict(grecip=grecip)
    return meta, percore, shared


def _build(meta, params):
    seg_ch, col0 = meta["seg_ch"], meta["col0"]
    gb_cols, g_col0 = meta["gb_cols"], meta["g_col0"]
    total_ch = meta["total_ch"]

    nc = bacc.Bacc("TRN2", target_bir_lowering=False, debug=False, num_devices=NCORES,
                   num_swdge_queues=4)

    def din(name, shape, dt):
        return nc.dram_tensor(name, shape, dt, kind="ExternalInput").ap()

    x_in = din("x_fm", [64, NPAD], F32)
    idx_in = din("idx", [128, total_ch * 8], I16)
    dstw_in = din("dstw", [128, total_ch], BF16)
    recip_in = din("recip_fm", [32, NPAD], F32)
    gh_in = din("gh", [128, NBLK * NUM_GRAPHS], BF16)
    grecip_in = din("grecip", [128, 2], F32)
    ident_in = din("ident", [128, 128], F32)
    iota_in = din("iota", [128, W], BF16)
    wp = {}
    for li, (ci, co, dd, pre) in enumerate(LAYERS):
        wp[f"wroot{li}"] = din(f"wroot{li}", [ci, co], F32)
        wp[f"wrel{li}"] = din(f"wrel{li}", [ci, co], F32)
        wp[f"gam{li}"] = din(f"gam{li}", [1, co], F32)
        wp[f"bet{li}"] = din(f"bet{li}", [1, co], F32)
    wlin1 = din("wlin1", [64, 64], F32)
    blin1 = din("blin1", [64, 1], F32)
    wlin2 = din("wlin2", [64, 10], F32)
    blin2 = din("blin2", [10, 1], F32)
    out_o = nc.dram_tensor("out", [NUM_GRAPHS, 10], F32, kind="ExternalOutput").ap()

    qn = [0]

    def next_q():
        qn[0] = (qn[0] + 1) % 4
        return qn[0]

    with tile.TileContext(nc) as tc:
        with tc.tile_pool(name="const", bufs=1) as cpool, \
             tc.tile_pool(name="nm", bufs=1) as nm_pool, \
             tc.tile_pool(name="stg", bufs=2) as stg_pool, \
             tc.tile_pool(name="idxp", bufs=2) as idx_pool, \
             tc.tile_pool(name="ohp", bufs=6) as oh_pool, \
             tc.tile_pool(name="ofp", bufs=3) as of_pool, \
             tc.tile_pool(name="sq", bufs=2) as sq_pool, \
             tc.tile_pool(name="ghp", bufs=3) as gh_pool, \
             tc.tile_pool(name="small", bufs=1) as sm_pool, \
             tc.tile_pool(name="aggps", bufs=2, space="PSUM") as agg_psp, \
             tc.tile_pool(name="nodeps", bufs=2, space="PSUM") as node_psp, \
             tc.tile_pool(name="trps", bufs=2, space="PSUM") as tr_psp, \
             tc.tile_pool(name="tr2ps", bufs=2, space="PSUM") as tr2_psp, \
             tc.tile_pool(name="dram", bufs=1, space="DRAM") as dram:

            cz = cpool.tile([128, 2], F32, name="constz")
            nc.vector.memset(cz[:, 0:1], 0.0)
            nc.vector.memset(cz[:, 1:2], EPS)
            nc.const_aps.aps[(F32, 0.0)] = cz[:, 0:1]
            nc.const_aps.aps[(F32, EPS)] = cz[:, 1:2]

            ident = cpool.tile([128, 128], F32, name="ident")
            nc.sync.dma_start(out=ident[:], in_=ident_in[:])
            iota_f = cpool.tile([128, W], BF16, name="iota_f")
            nc.sync.dma_start(out=iota_f[:], in_=iota_in[:])
            dstw_sb = cpool.tile([128, total_ch], BF16, name="dstw_sb")
            nc.sync.dma_start(out=dstw_sb[:], in_=dstw_in[:])
            grecip_sb = cpool.tile([128, 2], F32, name="grecip_sb")
            nc.sync.dma_start(out=grecip_sb[:], in_=grecip_in[:])
            wsb = {}
            for li, (ci, co, dd, pre) in enumerate(LAYERS):
                for nmk, shp in ((f"wroot{li}", [ci, co]), (f"wrel{li}", [ci, co]),
                                 (f"gam{li}", [1, co]), (f"bet{li}", [1, co])):
                    t = cpool.tile(shp, F32, name=f"c_{nmk}")
                    nc.sync.dma_start(out=t[:], in_=wp[nmk][:])
                    wsb[nmk] = t
            for nmk, src_ap, shp in (("wlin1", wlin1, [64, 64]), ("blin1", blin1, [64, 1]),
                                     ("wlin2", wlin2, [64, 10]), ("blin2", blin2, [10, 1])):
                t = cpool.tile(shp, F32, name=f"c_{nmk}")
                nc.sync.dma_start(out=t[:], in_=src_ap[:])
                wsb[nmk] = t

            # fm tensors live in DRAM; matmuls stream 512-node chunks
            agg_dram = dram.tile([32, NPAD], F32, name="agg_dram")
            h_dram = [dram.tile([64, NPAD], F32, name=f"hdram{i}", tag=f"hd{i % 2}")
                      for i in range(4)]

            # one 256B-strided table; buckets address it via int16 sign reach
            tbl = dram.tile([100352, TSTRIDE], BF16, name="tbl")

            def table_prep(li, agin_sb, dd):
                """agin_sb [128, NBLK, dd] bf16 (node-major shard) -> AG -> bucket tables."""
                agin = dram.tile([NSHARD, dd], BF16, name=f"agin{li}", tag=f"agin{dd}")
                full_blk = NSHARD // 128  # 97
                rem = NSHARD - full_blk * 128  # 84
                dst_ap = bass.AP(agin.tensor, agin.offset,
                                 [[dd, 128], [128 * dd, full_blk], [1, dd]])
                nc.sync.dma_start(out=dst_ap, in_=agin_sb[:, :full_blk, :])
                dst2 = bass.AP(agin.tensor, agin.offset + full_blk * 128 * dd,
                               [[dd, rem], [1, dd]])
                nc.sync.dma_start(out=dst2, in_=agin_sb[:rem, full_blk, :])
                packed = dram.tile([N, dd], BF16, name=f"packed{li}", tag=f"packed{dd}",
                                   addr_space="Shared")
                nc.gpsimd.collective_compute(
                    "AllGather", ALU.bypass, replica_groups=[list(range(NCORES))],
                    ins=[agin.opt()], outs=[packed.opt()])
                half = N // 2
                nc.sync.dma_start(out=tbl[:half, :dd], in_=packed[:half, :])
                nc.sync.dma_start(out=tbl[half:N, :dd], in_=packed[half:, :])

            def gather_aggregate(li, dd):
                """edge gather + onehot matmul accumulation -> agg_fm[:dd, :]."""
                for g in range(NGRP):
                    c0, c1 = int(g_col0[g]), int(g_col0[g + 1])
                    gcols = c1 - c0
                    idx_sl = idx_pool.tile([128, gcols * 8], I16, name=f"idx{li}_{g}", tag="idxsl")
                    nc.sync.dma_start(out=idx_sl[:], in_=idx_in[:, c0 * 8:c1 * 8])
                    stg = stg_pool.tile([128, gcols, dd], BF16, name=f"stg{li}_{g}", tag="stg")
                    # gathers per bucket
                    bc = 0
                    b_off = {}
                    for b in range(NBUCK):
                        nb_ch = int(gb_cols[g, b])
                        if nb_ch == 0:
                            continue
                        tbase = tbl[BBASE[b]:BBASE[b] + 1024, :dd]
                        # device limit: <=1024 indices (8 chunks) per dma_gather
                        for p0 in range(bc, bc + nb_ch, 8):
                            pch = min(8, bc + nb_ch - p0)
                            nc.gpsimd.dma_gather(
                                out_ap=stg[:, p0:p0 + pch, :],
                                in_ap=tbase,
                                idxs_ap=idx_sl[:, p0 * 8:(p0 + pch) * 8],
                                num_idxs=pch * 128,
                                num_idxs_reg=pch * 128,
                                elem_size=dd,
                                elem_step=TSTRIDE,
                                queue_num=next_q(),
                            )
                        b_off[b] = bc
                        bc += nb_ch
                    # group-local recip slice + agg buffer
                    rsl = of_pool.tile([32, GRP * W], F32, name=f"rsl{li}_{g}", tag="rsl")
                    nc.sync.dma_start(out=rsl[:], in_=recip_in[:, g * GRP * W:(g + 1) * GRP * W])
                    agsb = of_pool.tile([32, GRP * W], F32, name=f"agsb{li}_{g}", tag="agsb")
                    # per-tile psum accumulation
                    for t in range(g * GRP, (g + 1) * GRP):
                        tch = int(seg_ch[t].sum())
                        trel = (t - g * GRP) * W
                        if tch == 0:
                            nc.vector.memset(agsb[:dd, trel:trel + W], 0.0)
                            continue
                        aps = agg_psp.tile([32, W], F32, name=f"aggps{li}_{t}", tag="aggps")
                        first = True
                        done = 0
                        for b in range(NBUCK):
                            nch = int(seg_ch[t, b])
                            if nch == 0:
                                continue
                            rel = int(col0[t, b]) - c0
                            oh = oh_pool.tile([128, nch, W], BF16,
                                              name=f"oh{li}_{t}_{b}", tag="oh")
                            dsl = dstw_sb[:, c0 + rel:c0 + rel + nch]
                            iap = iota_f[:]
                            iota_b = bass.AP(iap.tensor, iap.offset,
                                             [iap.ap[0], [0, nch], [1, W]])
                            nc.vector.tensor_tensor(
                                out=oh[:], in0=dsl.to_broadcast([128, nch, W]),
                                in1=iota_b, op=ALU.is_equal)
                            for cc in range(nch):
                                done += 1
                                nc.tensor.matmul(
                                    out=aps[:dd, :],
                                    lhsT=stg[:, rel + cc, :],
                                    rhs=oh[:, cc, :],
                                    start=first, stop=(done == tch))
                                first = False
                        # agg = psum * recip
                        nc.vector.tensor_tensor(
                            out=agsb[:dd, trel:trel + W],
                            in0=aps[:dd, :],
                            in1=rsl[:dd, trel:trel + W],
                            op=ALU.mult)
                    nc.sync.dma_start(out=agg_dram[:dd, g * GRP * W:(g + 1) * GRP * W],
                                      in_=agsb[:dd, :])

            def node_compute(li, ci, co, dd, pre_flag, h_prev, pre_nm, e_nm):
                """dense matmuls + stats + transposes to pre_nm; returns stats tiles."""
                stats_s = sm_pool.tile([64, 25], F32, name=f"ss{li}", tag=f"ss{li}")
                stats_q = sm_pool.tile([64, 25], F32, name=f"sq{li}", tag=f"sq{li}")
                wroot = wsb[f"wroot{li}"]
                if pre_flag:
                    wrel_ap = ident[:16, :16]   # agg already projected (L1)
                else:
                    wrel_ap = wsb[f"wrel{li}"][:]
                nch = NPAD // NODE_CH            # 24 full + tail
                chunks = [(i * NODE_CH, NODE_CH) for i in range(nch)]
                if NPAD % NODE_CH:
                    chunks.append((nch * NODE_CH, NPAD % NODE_CH))
                for ci_, (off, ln) in enumerate(chunks):
                    hc = of_pool.tile([ci, NODE_CH], F32, name=f"hc{li}_{ci_}", tag="hc")
                    nc.sync.dma_start(out=hc[:, :ln], in_=h_prev[:ci, off:off + ln])
                    ac = of_pool.tile([32, NODE_CH], F32, name=f"ac{li}_{ci_}", tag="ac")
                    nc.sync.dma_start(out=ac[:dd, :ln], in_=agg_dram[:dd, off:off + ln])
                    ps = node_psp.tile([co, NODE_CH], F32, name=f"nps{li}_{ci_}", tag="nps")
                    nc.tensor.matmul(out=ps[:, :ln], lhsT=wroot[:],
                                     rhs=hc[:, :ln], start=True, stop=False)
                    nc.tensor.matmul(out=ps[:, :ln], lhsT=wrel_ap,
                                     rhs=ac[:dd, :ln], start=False, stop=True)
                    of = of_pool.tile([co, NODE_CH], F32, name=f"of{li}_{ci_}", tag="of")
                    nc.vector.tensor_copy(out=of[:, :ln], in_=ps[:, :ln])
                    real = max(0, min(NSHARD - off, ln))
                    if real:
                        nc.vector.tensor_reduce(out=stats_s[:co, ci_:ci_ + 1],
                                                in_=of[:, :real], axis=mybir.AxisListType.X,
                                                op=ALU.add)
                        sqs = sq_pool.tile([co, NODE_CH], F32, name=f"sqs{li}_{ci_}", tag="sqs")
                        nc.scalar.activation(out=sqs[:, :real], in_=of[:, :real],
                                             func=AF.Square,
                                             accum_out=stats_q[:co, ci_:ci_ + 1])
                    for j in range(ln // 128):
                        blk = (off + j * 128) // 128
                        tp = tr_psp.tile([128, 64], F32, name=f"tp{li}_{ci_}_{j}", tag="trp")
                        nc.tensor.transpose(out=tp[:, :co], in_=of[:, j * 128:(j + 1) * 128],
                                            identity=ident[:co, :co])
                        nc.vector.tensor_copy(out=pre_nm[:, blk, :co], in_=tp[:, :co])
                return stats_s, stats_q, len(chunks)

            def bn_elu(li, co, pre_nm, e_nm, stats_s, stats_q, nchunks):
                """stats allreduce -> affine params -> NM affine+ELU in pre_nm."""
                srow = sm_pool.tile([64, 2], F32, name=f"st{li}", tag=f"st{li}")
                nc.vector.tensor_reduce(out=srow[:co, 0:1], in_=stats_s[:co, :nchunks],
                                        axis=mybir.AxisListType.X, op=ALU.add)
                nc.vector.tensor_reduce(out=srow[:co, 1:2], in_=stats_q[:co, :nchunks],
                                        axis=mybir.AxisListType.X, op=ALU.add)
                arin = dram.tile([co, 2], F32, name=f"arin{li}", tag=f"arin{li}")
                arout = dram.tile([co, 2], F32, name=f"arout{li}", tag=f"arout{li}",
                                  addr_space="Shared")
                nc.sync.dma_start(out=arin[:], in_=srow[:co, :])
                nc.gpsimd.collective_compute(
                    "AllReduce", ALU.add, replica_groups=[list(range(NCORES))],
                    ins=[arin.opt()], outs=[arout.opt()])
                # rows [1, co]
                rows = sm_pool.tile([1, 8 * 64], F32, name=f"rw{li}", tag=f"rw{li}")
                mean_r, q_r = rows[:, 0:co], rows[:, 64:64 + co]
                var_r, a_r = rows[:, 128:128 + co], rows[:, 192:192 + co]
                b_r, t_r = rows[:, 256:256 + co], rows[:, 320:320 + co]
                nc.sync.dma_start(out=mean_r, in_=bass.AP(arout.tensor, arout.offset, [[0, 1], [2, co]]))
                nc.sync.dma_start(out=q_r, in_=bass.AP(arout.tensor, arout.offset + 1, [[0, 1], [2, co]]))
                nc.vector.tensor_scalar_mul(out=mean_r, in0=mean_r, scalar1=1.0 / N)
                nc.vector.tensor_scalar_mul(out=q_r, in0=q_r, scalar1=1.0 / N)
                nc.vector.tensor_tensor(out=var_r, in0=mean_r, in1=mean_r, op=ALU.mult)
                nc.vector.tensor_tensor(out=var_r, in0=q_r, in1=var_r, op=ALU.subtract)
                nc.scalar.activation(out=var_r, in_=var_r, func=AF.Sqrt, bias=EPS)
                nc.vector.reciprocal(out=var_r, in_=var_r)
                gam = cpool_row(li)
                nc.vector.tensor_tensor(out=a_r, in0=var_r, in1=wsb[f"gam{li}"][:], op=ALU.mult)
                nc.vector.tensor_tensor(out=t_r, in0=mean_r, in1=a_r, op=ALU.mult)
                nc.vector.tensor_tensor(out=b_r, in0=wsb[f"bet{li}"][:], in1=t_r, op=ALU.subtract)
                del gam
                abd = dram.tile([2, 64], F32, name=f"abd{li}", tag=f"abd{li}")
                nc.sync.dma_start(out=abd[0:1, :co], in_=a_r)
                nc.sync.dma_start(out=abd[1:2, :co], in_=b_r)
                ab_bc = sm_pool.tile([128, 2, 64], F32, name=f"abbc{li}", tag=f"abbc{li}")
                nc.sync.dma_start(out=ab_bc[:],
                                  in_=bass.AP(abd.tensor, abd.offset, [[0, 128], [64, 2], [1, 64]]))
                # NM affine + ELU (in place in pre_nm)
                av = ab_bc[:, 0, :co]
                bv = ab_bc[:, 1, :co]
                a_b = bass.AP(av.tensor, av.offset, [av.ap[0], [0, NBLK], [1, co]])
                b_b = bass.AP(bv.tensor, bv.offset, [bv.ap[0], [0, NBLK], [1, co]])
                pnm = pre_nm[:, :, :co]
                nc.vector.tensor_tensor(out=pnm, in0=pnm, in1=a_b, op=ALU.mult)
                nc.vector.tensor_tensor(out=pnm, in0=pnm, in1=b_b, op=ALU.add)
                enm = e_nm[:, :, :co]
                nc.vector.tensor_scalar_min(out=enm, in0=pnm, scalar1=0.0)
                nc.scalar.activation(out=enm, in_=enm, func=AF.Exp)
                nc.vector.tensor_scalar_add(out=enm, in0=enm, scalar1=-1.0)
                nc.vector.tensor_tensor(out=pnm, in0=pnm, in1=enm, op=ALU.max)

            def cpool_row(li):
                return None

            # ---------------- layer loop ----------------
            h_prev = x_in[:]
            pre_nm = nm_pool.tile([128, NBLK, 64], F32, name="pre_nm")
            e_nm = nm_pool.tile([128, NBLK, 64], F32, name="e_nm")

            for li, (ci, co, dd, pre_flag) in enumerate(LAYERS):
                # --- table prep
                agin_sb = nm_pool.tile([128, NBLK, dd], BF16, name=f"aginsb{li}", tag="aginsb")
                if li == 0:
                    # xp1 = x @ w_rel1, node-major bf16
                    nch = NPAD // NODE_CH
                    chunks = [(i * NODE_CH, NODE_CH) for i in range(nch)]
                    if NPAD % NODE_CH:
                        chunks.append((nch * NODE_CH, NPAD % NODE_CH))
                    for ci_, (off, ln) in enumerate(chunks):
                        xc = of_pool.tile([64, NODE_CH], F32, name=f"xc{ci_}", tag="hc")
                        nc.sync.dma_start(out=xc[:, :ln], in_=x_in[:, off:off + ln])
                        ps = node_psp.tile([16, NODE_CH], F32, name=f"xps{ci_}", tag="nps")
                        nc.tensor.matmul(out=ps[:, :ln], lhsT=wsb["wrel0"][:],
                                         rhs=xc[:, :ln], start=True, stop=True)
                        of = of_pool.tile([16, NODE_CH], F32, name=f"xof{ci_}", tag="of")
                        nc.vector.tensor_copy(out=of[:, :ln], in_=ps[:, :ln])
                        for j in range(ln // 128):
                            blk = (off + j * 128) // 128
                            tp = tr_psp.tile([128, 64], F32, name=f"xtp{ci_}_{j}", tag="trp")
                            nc.tensor.transpose(out=tp[:, :16], in_=of[:, j * 128:(j + 1) * 128],
                                                identity=ident[:16, :16])
                            nc.vector.tensor_copy(out=agin_sb[:, blk, :], in_=tp[:, :16])
                else:
                    nc.vector.tensor_copy(out=agin_sb[:], in_=pre_nm[:, :, :dd])
                table_prep(li, agin_sb, dd)

                # --- gather + aggregate
                gather_aggregate(li, dd)

                # --- dense node compute
                stats_s, stats_q, nchunks = node_compute(li, ci, co, dd, pre_flag,
                                                         h_prev, pre_nm, e_nm)
                bn_elu(li, co, pre_nm, e_nm, stats_s, stats_q, nchunks)

                # h_fm via transposes back -> DRAM (skip for last layer: pooling
                # consumes the node-major form directly)
                if li < len(LAYERS) - 1:
                    for ch4 in range((NBLK + 3) // 4):
                        blks = range(ch4 * 4, min((ch4 + 1) * 4, NBLK))
                        tb_sb = of_pool.tile([64, NODE_CH], F32, name=f"tb{li}_{ch4}", tag="tbsb")
                        for j, blk in enumerate(blks):
                            tp2 = tr2_psp.tile([64, 128], F32, name=f"tb{li}_{blk}", tag="tr2")
                            nc.tensor.transpose(out=tp2[:co, :], in_=pre_nm[:, blk, :co],
                                                identity=ident[:128, :128])
                            nc.vector.tensor_copy(out=tb_sb[:co, j * 128:(j + 1) * 128],
                                                  in_=tp2[:co, :])
                        ln4 = len(blks) * 128
                        nc.sync.dma_start(
                            out=h_dram[li][:co, ch4 * NODE_CH:ch4 * NODE_CH + ln4],
                            in_=tb_sb[:co, :ln4])
                    h_prev = h_dram[li][:]

            # ---------------- pooling + MLP + log_softmax ----------------
            pool_ps = [node_psp.tile([128, 64], F32, name=f"pps{h}", tag="nps") for h in range(2)]
            for blk in range(NBLK):
                ghc = gh_pool.tile([128, NUM_GRAPHS], BF16, name=f"ghc{blk}", tag="ghc")
                nc.sync.dma_start(out=ghc[:], in_=gh_in[:, blk * NUM_GRAPHS:(blk + 1) * NUM_GRAPHS])
                hbf = gh_pool.tile([128, 64], BF16, name=f"hbf{blk}", tag="hbf")
                nc.vector.tensor_copy(out=hbf[:], in_=pre_nm[:, blk, :64])
                for h in range(2):
                    nc.tensor.matmul(out=pool_ps[h][:], lhsT=ghc[:, h * 128:(h + 1) * 128],
                                     rhs=hbf[:], start=(blk == 0), stop=(blk == NBLK - 1))
            pooled_nm = sm_pool.tile([128, 2, 64], F32, name="pooled_nm")
            for h in range(2):
                nc.vector.tensor_copy(out=pooled_nm[:, h, :], in_=pool_ps[h][:])
            par_in = dram.tile([NUM_GRAPHS, 64], F32, name="par_in")
            par_out = dram.tile([NUM_GRAPHS, 64], F32, name="par_out", addr_space="Shared")
            par_ap = bass.AP(par_in.tensor, par_in.offset, [[64, 128], [128 * 64, 2], [1, 64]])
            nc.sync.dma_start(out=par_ap, in_=pooled_nm[:])
            nc.gpsimd.collective_compute(
                "AllReduce", ALU.add, replica_groups=[list(range(NCORES))],
                ins=[par_in.opt()], outs=[par_out.opt()])
            nc.sync.dma_start(out=pooled_nm[:],
                              in_=bass.AP(par_out.tensor, par_out.offset,
                                          [[64, 128], [128 * 64, 2], [1, 64]]))
            pooled_fm = sm_pool.tile([64, NUM_GRAPHS], F32, name="pooled_fm")
            for h in range(2):
                nc.scalar.activation(out=pooled_nm[:, h, :], in_=pooled_nm[:, h, :],
                                     func=AF.Copy, scale=grecip_sb[:, h:h + 1])
                tp2 = tr2_psp.tile([64, 128], F32, name=f"ptr{h}", tag="tr2")
                nc.tensor.transpose(out=tp2[:], in_=pooled_nm[:, h, :],
                                    identity=ident[:])
                nc.vector.tensor_copy(out=pooled_fm[:, h * 128:(h + 1) * 128], in_=tp2[:])
            z_ps = node_psp.tile([64, NUM_GRAPHS], F32, name="z_ps", tag="nps")
            nc.tensor.matmul(out=z_ps[:], lhsT=wsb["wlin1"][:], rhs=pooled_fm[:],
                             start=True, stop=True)
            z_fm = sm_pool.tile([64, NUM_GRAPHS], F32, name="z_fm")
            nc.scalar.activation(out=z_fm[:], in_=z_ps[:], func=AF.Relu,
                                 bias=wsb["blin1"][:])
            lg_ps = node_psp.tile([10, NUM_GRAPHS], F32, name="lg_ps", tag="nps")
            nc.tensor.matmul(out=lg_ps[:], lhsT=wsb["wlin2"][:], rhs=z_fm[:],
                             start=True, stop=True)
            logits_fm = sm_pool.tile([10, NUM_GRAPHS], F32, name="logits_fm")
            nc.scalar.activation(out=logits_fm[:], in_=lg_ps[:], func=AF.Identity,
                                 bias=wsb["blin2"][:])
            lnm = sm_pool.tile([128, 2, 10], F32, name="lnm")
            mrow = sm_pool.tile([128, 4], F32, name="mrow")
            for h in range(2):
                tp3 = tr_psp.tile([128, 64], F32, name=f"ltr{h}", tag="trp")
                nc.tensor.transpose(out=tp3[:, :10], in_=logits_fm[:, h * 128:(h + 1) * 128],
                                    identity=ident[:10, :10])
                nc.vector.tensor_copy(out=lnm[:, h, :], in_=tp3[:, :10])
                nc.vector.tensor_reduce(out=mrow[:, h:h + 1], in_=lnm[:, h, :],
                                        axis=mybir.AxisListType.X, op=ALU.max)
                nc.vector.tensor_scalar(out=lnm[:, h, :], in0=lnm[:, h, :],
                                        scalar1=mrow[:, h:h + 1], scalar2=None,
                                        op0=ALU.subtract)
                esb = sm_pool.tile([128, 10], F32, name=f"esb{h}", tag="esb")
                nc.scalar.activation(out=esb[:], in_=lnm[:, h, :], func=AF.Exp)
                nc.vector.tensor_reduce(out=mrow[:, 2 + h:3 + h], in_=esb[:],
                                        axis=mybir.AxisListType.X, op=ALU.add)
                nc.scalar.activation(out=mrow[:, 2 + h:3 + h], in_=mrow[:, 2 + h:3 + h],
                                     func=AF.Ln)
                nc.vector.tensor_scalar(out=lnm[:, h, :], in0=lnm[:, h, :],
                                        scalar1=mrow[:, 2 + h:3 + h], scalar2=None,
                                        op0=ALU.subtract)
                nc.sync.dma_start(out=out_o[h * 128:(h + 1) * 128, :], in_=lnm[:, h, :])

    nc.compile()
    return nc


def kernel(**inputs):
    x = inputs["x"]
    edge_index = inputs["edge_index"]
    batch = inputs["batch"]
    meta, percore, shared = _preprocess(x, edge_index, batch)

    nc = _build(meta, inputs)

    ident = np.eye(128, dtype=np.float32)
    iota = np.tile(np.arange(W, dtype=np.float32), (128, 1)).astype(ml_dtypes.bfloat16)
    in_maps = []
    for k in range(NCORES):
        m = {
            "x_fm": percore["x_fm"][k],
            "idx": percore["idx"][k],
            "dstw": percore["dstw"][k],
            "recip_fm": percore["recip_fm"][k],
            "gh": percore["gh"][k],
            "grecip": shared["grecip"],
            "ident": ident,
            "iota": iota,
            "wlin1": np.asarray(inputs["w_lin1"], np.float32),
            "blin1": np.asarray(inputs["b_lin1"], np.float32).reshape(64, 1),
            "wlin2": np.asarray(inputs["w_lin2"], np.float32),
            "blin2": np.asarray(inputs["b_lin2"], np.float32).reshape(10, 1),
        }
        for li in range(4):
            m[f"wroot{li}"] = np.asarray(inputs[f"w_root{li + 1}"], np.float32)
            m[f"wrel{li}"] = np.asarray(inputs[f"w_rel{li + 1}"], np.float32)
            m[f"gam{li}"] = np.asarray(inputs[f"g{li + 1}"], np.float32).reshape(1, -1)
            m[f"bet{li}"] = np.asarray(inputs[f"be{li + 1}"], np.float32).reshape(1, -1)
        in_maps.append(m)

    global _LAST
    _LAST = (nc, in_maps)
    res = run_bass_kernel_spmd(nc, in_maps, list(range(NCORES)))
    return np.asarray(res.results[0]["out"], dtype=np.float32)


_LAST = None


def rerun():
    """Re-execute the last compiled kernel (for timing)."""
    import time
    nc, in_maps = _LAST
    t0 = time.time()
    run_bass_kernel_spmd(nc, in_maps, list(range(NCORES)))
    return time.time() - t0


if __name__ == "__main__":
    import reference
    ins = {k: np.asarray(v) for k, v in reference.setup_inputs().items()}
    out = kernel(**ins)
    print("kernel out", out.shape, out.dtype, out[:2])



# revision 10
# speedup vs baseline: 243.3192x; 243.3192x over previous
"""GNN message-passing kernel (4x GraphConv + BN + ELU, mean-pool, MLP, log_softmax)
for 8 Trainium2 NeuronCores.

Strategy:
  - Shard nodes (and their incident in-edges) across 8 cores by contiguous node
    ranges (batch stays sorted -> graph segments stay local-ish).
  - Aggregation: dma_gather (bf16, small elems) of source-node features from a
    256B-strided replicated table, then one-hot matmul accumulation into PSUM
    per 64-node destination tile.  Host precomputes all index structures.
  - Feature-major (fm) layout [C, nodes] for matmuls/stats; node-major (nm)
    [128, blocks, C] for BN-affine+ELU; PE transposes convert.
  - Collectives: AllReduce for BN stats + pooled logits; AllGather for the
    per-layer node-feature tables.
"""
import inspect
import textwrap

import ml_dtypes
import numpy as np

import concourse.bacc as bacc
import concourse.bass as bass
import concourse.mybir as mybir
import concourse.tile as tile
from concourse.bass_utils import run_bass_kernel_spmd

# ---- relax dma_gather's 256B elem restriction to 32B (ucode supports it; the
# stride stays 256B-quantized which we honor with 128-bf16-elem table rows).
_src = inspect.getsource(bass.BassGpSimd.dma_gather)
_src = _src.replace("elem_size_bytes % 256 == 0", "elem_size_bytes % 32 == 0")
_src = "def dma_gather" + _src.split("def dma_gather", 1)[1]
_ns = dict(bass.__dict__)
exec(textwrap.dedent(_src), _ns)
bass.BassGpSimd.dma_gather = _ns["dma_gather"]

F32 = mybir.dt.float32
BF16 = mybir.dt.bfloat16
I16 = mybir.dt.int16
AF = mybir.ActivationFunctionType
ALU = mybir.AluOpType

N = 100000
E = 3200000
NCORES = 8
NSHARD = N // NCORES          # 12500
NBLK = (NSHARD + 127) // 128  # 98
NPAD = NBLK * 128             # 12544
NUM_GRAPHS = 256
EPS = 1e-5
W = 128                       # dst-tile width / onehot width
NTILES = (NSHARD + W - 1) // W   # 98
# int16 idx used as non-negative offset from a per-bucket base row
BBASE = [0, 32768, 65536, 98304]
NBUCK = 4
GRP = 7                       # tiles per gather group
NGRP = NTILES // GRP          # 14
NODE_CH = 512                 # fm node-chunk for the dense matmuls
TSTRIDE = 128                 # bf16 elems per table row = 256B

# (C_in, C_out, D_agg, project_before)
LAYERS = [(64, 16, 16, True), (16, 32, 16, False), (32, 32, 32, False), (32, 64, 32, False)]


def _preprocess(x, edge_index, batch):
    src = np.asarray(edge_index[0], dtype=np.int64)
    dst = np.asarray(edge_index[1], dtype=np.int64)
    batch = np.asarray(batch, dtype=np.int64)
    x = np.asarray(x, dtype=np.float32)

    core = dst // NSHARD
    dstloc = dst - core * NSHARD
    tl = dstloc // W
    wloc = dstloc - tl * W
    bk = src // 32768

    # per (core, tile, bucket) edge counts -> uniform segment lengths
    key = (core * NTILES + tl) * NBUCK + bk
    cnt = np.bincount(key, minlength=NCORES * NTILES * NBUCK).reshape(NCORES, NTILES, NBUCK)
    seg = cnt.max(axis=0)
    seg = ((seg + 127) // 128) * 128 * (seg > 0)       # [NTILES, NBUCK]

    # chunk layout: for g in groups: for b in buckets: for t in group: seg(t,b)
    seg_ch = seg // 128
    # global column index of each (t,b) block start
    col0 = np.zeros((NTILES, NBUCK), dtype=np.int64)
    gb_cols = np.zeros((NGRP, NBUCK), dtype=np.int64)   # cols per (group,bucket)
    g_col0 = np.zeros(NGRP + 1, dtype=np.int64)
    c = 0
    for g in range(NGRP):
        g_col0[g] = c
        for b in range(NBUCK):
            for t in range(g * GRP, (g + 1) * GRP):
                col0[t, b] = c
                c += seg_ch[t, b]
            gb_cols[g, b] = c - (col0[g * GRP, b])
    g_col0[NGRP] = c
    total_ch = c
    total_e = total_ch * 128

    # per-core edge placement
    order = np.argsort(core * np.int64(NTILES * NBUCK) + tl * NBUCK + bk, kind="stable")
    s_src, s_tl, s_bk, s_w, s_core = src[order], tl[order], bk[order], wloc[order], core[order]

    idx16 = np.zeros((NCORES, total_e), dtype=np.int16)
    dstw = np.full((NCORES, total_e), -1.0, dtype=np.float32)
    pos = 0
    for k in range(NCORES):
        for t in range(NTILES):
            for b in range(NBUCK):
                n = cnt[k, t, b]
                if n:
                    sl = slice(pos, pos + n)
                    base = col0[t, b] * 128
                    p = np.arange(n)
                    idx16[k, base + p] = (s_src[sl] - BBASE[b]).astype(np.int16)
                    dstw[k, base + p] = s_w[sl]
                    pos += n
    assert pos == E

    # wrapped (16) idx layout: position i -> [i%16, i//16]; replicated x8 on
    # device at load time (saves 7/8 of the idx upload bytes).
    idx_w = idx16.reshape(NCORES, total_e // 16, 16).transpose(0, 2, 1)  # [NCORES,16,tot/16]
    idx_rep = np.ascontiguousarray(idx_w)
    # dstw [core, 128, total_ch]: position i=(c*128+p) -> [p, c]
    dstw_pc = dstw.reshape(NCORES, total_ch, 128).transpose(0, 2, 1).astype(ml_dtypes.bfloat16)

    # x shards, fm layout
    x_fm = np.zeros((NCORES, 64, NPAD), dtype=np.float32)
    for k in range(NCORES):
        x_fm[k, :, :NSHARD] = x[k * NSHARD:(k + 1) * NSHARD].T

    # in-degree reciprocal, single row [1, NPAD] (replicated to 32 partitions
    # on device at load time)
    indeg = np.bincount(dst, minlength=N).astype(np.float32)
    recip = 1.0 / np.maximum(indeg, 1.0)
    recip_1r = np.zeros((NCORES, 1, NPAD), dtype=np.float32)
    for k in range(NCORES):
        recip_1r[k, 0, :NSHARD] = recip[k * NSHARD:(k + 1) * NSHARD]

    # graph ids, node-major [128, NBLK] bf16 (one-hot built on device);
    # padding rows get id 300 (outside [0,256) -> all-zero one-hot row)
    bid = np.full((NCORES, NPAD), 300.0, dtype=np.float32)
    for k in range(NCORES):
        bid[k, :NSHARD] = batch[k * NSHARD:(k + 1) * NSHARD]
    bid = bid.reshape(NCORES, NBLK, 128).transpose(0, 2, 1)  # [NCORES,128,NBLK]
    bid = np.ascontiguousarray(bid).astype(ml_dtypes.bfloat16)

    gcnt = np.bincount(batch, minlength=NUM_GRAPHS).astype(np.float32)
    grecip = (1.0 / np.maximum(gcnt, 1.0)).reshape(2, 128).T.astype(np.float32)  # [128,2]

    meta = dict(seg_ch=seg_ch, col0=col0, gb_cols=gb_cols, g_col0=g_col0,
                total_ch=int(total_ch))
    percore = dict(idx=idx_rep, dstw=dstw_pc, x_fm=x_fm, recip_1r=recip_1r, bid=bid)
    shared = dict(grecip=grecip)
    return meta, percore, shared


def _build(meta, params):
    seg_ch, col0 = meta["seg_ch"], meta["col0"]
    gb_cols, g_col0 = meta["gb_cols"], meta["g_col0"]
    total_ch = meta["total_ch"]

    nc = bacc.Bacc("TRN2", target_bir_lowering=False, debug=False, num_devices=NCORES,
                   num_swdge_queues=4)

    def din(name, shape, dt):
        return nc.dram_tensor(name, shape, dt, kind="ExternalInput").ap()

    x_in = din("x_fm", [64, NPAD], F32)
    idx_in = din("idx", [16, total_ch * 8], I16)
    dstw_in = din("dstw", [128, total_ch], BF16)
    recip_in = din("recip_1r", [1, NPAD], F32)
    bid_in = din("bid", [128, NBLK], BF16)
    iota256_in = din("iota256", [128, NUM_GRAPHS], BF16)
    grecip_in = din("grecip", [128, 2], F32)
    ident_in = din("ident", [128, 128], F32)
    iota_in = din("iota", [128, W], BF16)
    wp = {}
    for li, (ci, co, dd, pre) in enumerate(LAYERS):
        wp[f"wroot{li}"] = din(f"wroot{li}", [ci, co], F32)
        wp[f"wrel{li}"] = din(f"wrel{li}", [ci, co], F32)
        wp[f"gam{li}"] = din(f"gam{li}", [1, co], F32)
        wp[f"bet{li}"] = din(f"bet{li}", [1, co], F32)
    wlin1 = din("wlin1", [64, 64], F32)
    blin1 = din("blin1", [64, 1], F32)
    wlin2 = din("wlin2", [64, 10], F32)
    blin2 = din("blin2", [10, 1], F32)
    out_o = nc.dram_tensor("out", [NUM_GRAPHS, 10], F32, kind="ExternalOutput").ap()

    qn = [0]

    def next_q():
        qn[0] = (qn[0] + 1) % 4
        return qn[0]

    with tile.TileContext(nc) as tc:
        with tc.tile_pool(name="const", bufs=1) as cpool, \
             tc.tile_pool(name="nm", bufs=1) as nm_pool, \
             tc.tile_pool(name="stg", bufs=2) as stg_pool, \
             tc.tile_pool(name="idxp", bufs=2) as idx_pool, \
             tc.tile_pool(name="ohp", bufs=6) as oh_pool, \
             tc.tile_pool(name="ofp", bufs=3) as of_pool, \
             tc.tile_pool(name="sq", bufs=2) as sq_pool, \
             tc.tile_pool(name="ghp", bufs=3) as gh_pool, \
             tc.tile_pool(name="small", bufs=1) as sm_pool, \
             tc.tile_pool(name="aggps", bufs=2, space="PSUM") as agg_psp, \
             tc.tile_pool(name="nodeps", bufs=2, space="PSUM") as node_psp, \
             tc.tile_pool(name="trps", bufs=2, space="PSUM") as tr_psp, \
             tc.tile_pool(name="tr2ps", bufs=2, space="PSUM") as tr2_psp, \
             tc.tile_pool(name="dram", bufs=1, space="DRAM") as dram:

            cz = cpool.tile([128, 2], F32, name="constz")
            nc.vector.memset(cz[:, 0:1], 0.0)
            nc.vector.memset(cz[:, 1:2], EPS)
            nc.const_aps.aps[(F32, 0.0)] = cz[:, 0:1]
            nc.const_aps.aps[(F32, EPS)] = cz[:, 1:2]

            ident = cpool.tile([128, 128], F32, name="ident")
            nc.sync.dma_start(out=ident[:], in_=ident_in[:])
            iota_f = cpool.tile([128, W], BF16, name="iota_f")
            nc.sync.dma_start(out=iota_f[:], in_=iota_in[:])
            bid_sb = cpool.tile([128, NBLK], BF16, name="bid_sb")
            nc.sync.dma_start(out=bid_sb[:], in_=bid_in[:])
            iota256 = cpool.tile([128, NUM_GRAPHS], BF16, name="iota256")
            nc.sync.dma_start(out=iota256[:], in_=iota256_in[:])
            dstw_sb = cpool.tile([128, total_ch], BF16, name="dstw_sb")
            nc.sync.dma_start(out=dstw_sb[:], in_=dstw_in[:])
            grecip_sb = cpool.tile([128, 2], F32, name="grecip_sb")
            nc.sync.dma_start(out=grecip_sb[:], in_=grecip_in[:])
            wsb = {}
            for li, (ci, co, dd, pre) in enumerate(LAYERS):
                for nmk, shp in ((f"wroot{li}", [ci, co]), (f"wrel{li}", [ci, co]),
                                 (f"gam{li}", [1, co]), (f"bet{li}", [1, co])):
                    t = cpool.tile(shp, F32, name=f"c_{nmk}")
                    nc.sync.dma_start(out=t[:], in_=wp[nmk][:])
                    wsb[nmk] = t
            for nmk, src_ap, shp in (("wlin1", wlin1, [64, 64]), ("blin1", blin1, [64, 1]),
                                     ("wlin2", wlin2, [64, 10]), ("blin2", blin2, [10, 1])):
                t = cpool.tile(shp, F32, name=f"c_{nmk}")
                nc.sync.dma_start(out=t[:], in_=src_ap[:])
                wsb[nmk] = t

            # fm tensors live in DRAM; matmuls stream 512-node chunks
            agg_dram = dram.tile([32, NPAD], F32, name="agg_dram")
            h_dram = [dram.tile([64, NPAD], F32, name=f"hdram{i}", tag=f"hd{i % 2}")
                      for i in range(4)]

            # one 256B-strided table; buckets address it via int16 sign reach
            tbl = dram.tile([100352, TSTRIDE], BF16, name="tbl")

            def table_prep(li, agin_sb, dd):
                """agin_sb [128, NBLK, dd] bf16 (node-major shard) -> AG -> bucket tables."""
                agin = dram.tile([NSHARD, dd], BF16, name=f"agin{li}", tag=f"agin{dd}")
                full_blk = NSHARD // 128  # 97
                rem = NSHARD - full_blk * 128  # 84
                dst_ap = bass.AP(agin.tensor, agin.offset,
                                 [[dd, 128], [128 * dd, full_blk], [1, dd]])
                nc.sync.dma_start(out=dst_ap, in_=agin_sb[:, :full_blk, :])
                dst2 = bass.AP(agin.tensor, agin.offset + full_blk * 128 * dd,
                               [[dd, rem], [1, dd]])
                nc.sync.dma_start(out=dst2, in_=agin_sb[:rem, full_blk, :])
                packed = dram.tile([N, dd], BF16, name=f"packed{li}", tag=f"packed{dd}",
                                   addr_space="Shared")
                nc.gpsimd.collective_compute(
                    "AllGather", ALU.bypass, replica_groups=[list(range(NCORES))],
                    ins=[agin.opt()], outs=[packed.opt()])
                half = N // 2
                nc.sync.dma_start(out=tbl[:half, :dd], in_=packed[:half, :])
                nc.sync.dma_start(out=tbl[half:N, :dd], in_=packed[half:, :])

            def gather_aggregate(li, dd):
                """edge gather + onehot matmul accumulation -> agg_fm[:dd, :]."""
                for g in range(NGRP):
                    c0, c1 = int(g_col0[g]), int(g_col0[g + 1])
                    gcols = c1 - c0
                    idx_sl = idx_pool.tile([128, gcols * 8], I16, name=f"idx{li}_{g}", tag="idxsl")
                    # replicate the [16, cols] wrapped idx block to 128
                    # partitions (8 copies) while loading
                    idx_src = bass.AP(idx_in.tensor, idx_in.offset + c0 * 8,
                                      [[0, 8], [total_ch * 8, 16], [1, gcols * 8]])
                    nc.sync.dma_start(out=idx_sl[:], in_=idx_src)
                    stg = stg_pool.tile([128, gcols, dd], BF16, name=f"stg{li}_{g}", tag="stg")
                    # gathers per bucket
                    bc = 0
                    b_off = {}
                    for b in range(NBUCK):
                        nb_ch = int(gb_cols[g, b])
                        if nb_ch == 0:
                            continue
                        tbase = tbl[BBASE[b]:BBASE[b] + 1024, :dd]
                        # device limit: <=1024 indices (8 chunks) per dma_gather
                        for p0 in range(bc, bc + nb_ch, 8):
                            pch = min(8, bc + nb_ch - p0)
                            nc.gpsimd.dma_gather(
                                out_ap=stg[:, p0:p0 + pch, :],
                                in_ap=tbase,
                                idxs_ap=idx_sl[:, p0 * 8:(p0 + pch) * 8],
                                num_idxs=pch * 128,
                                num_idxs_reg=pch * 128,
                                elem_size=dd,
                                elem_step=TSTRIDE,
                                queue_num=next_q(),
                            )
                        b_off[b] = bc
                        bc += nb_ch
                    # group-local recip slice (single DRAM row replicated to
                    # 32 partitions while loading) + agg buffer
                    rsl = of_pool.tile([32, GRP * W], F32, name=f"rsl{li}_{g}", tag="rsl")
                    rsl_src = bass.AP(recip_in.tensor, recip_in.offset + g * GRP * W,
                                      [[0, 32], [1, GRP * W]])
                    nc.sync.dma_start(out=rsl[:], in_=rsl_src)
                    agsb = of_pool.tile([32, GRP * W], F32, name=f"agsb{li}_{g}", tag="agsb")
                    # per-tile psum accumulation
                    for t in range(g * GRP, (g + 1) * GRP):
                        tch = int(seg_ch[t].sum())
                        trel = (t - g * GRP) * W
                        if tch == 0:
                            nc.vector.memset(agsb[:dd, trel:trel + W], 0.0)
                            continue
                        aps = agg_psp.tile([32, W], F32, name=f"aggps{li}_{t}", tag="aggps")
                        first = True
                        done = 0
                        for b in range(NBUCK):
                            nch = int(seg_ch[t, b])
                            if nch == 0:
                                continue
                            rel = int(col0[t, b]) - c0
                            oh = oh_pool.tile([128, nch, W], BF16,
                                              name=f"oh{li}_{t}_{b}", tag="oh")
                            dsl = dstw_sb[:, c0 + rel:c0 + rel + nch]
                            iap = iota_f[:]
                            iota_b = bass.AP(iap.tensor, iap.offset,
                                             [iap.ap[0], [0, nch], [1, W]])
                            nc.vector.tensor_tensor(
                                out=oh[:], in0=dsl.to_broadcast([128, nch, W]),
                                in1=iota_b, op=ALU.is_equal)
                            for cc in range(nch):
                                done += 1
                                nc.tensor.matmul(
                                    out=aps[:dd, :],
                                    lhsT=stg[:, rel + cc, :],
                                    rhs=oh[:, cc, :],
                                    start=first, stop=(done == tch))
                                first = False
                        # agg = psum * recip
                        nc.vector.tensor_tensor(
                            out=agsb[:dd, trel:trel + W],
                            in0=aps[:dd, :],
                            in1=rsl[:dd, trel:trel + W],
                            op=ALU.mult)
                    nc.sync.dma_start(out=agg_dram[:dd, g * GRP * W:(g + 1) * GRP * W],
                                      in_=agsb[:dd, :])

            def node_compute(li, ci, co, dd, pre_flag, h_prev, pre_nm, e_nm):
                """dense matmuls + stats + transposes to pre_nm; returns stats tiles."""
                stats_s = sm_pool.tile([64, 25], F32, name=f"ss{li}", tag=f"ss{li}")
                stats_q = sm_pool.tile([64, 25], F32, name=f"sq{li}", tag=f"sq{li}")
                wroot = wsb[f"wroot{li}"]
                if pre_flag:
                    wrel_ap = ident[:16, :16]   # agg already projected (L1)
                else:
                    wrel_ap = wsb[f"wrel{li}"][:]
                nch = NPAD // NODE_CH            # 24 full + tail
                chunks = [(i * NODE_CH, NODE_CH) for i in range(nch)]
                if NPAD % NODE_CH:
                    chunks.append((nch * NODE_CH, NPAD % NODE_CH))
                for ci_, (off, ln) in enumerate(chunks):
                    hc = of_pool.tile([ci, NODE_CH], F32, name=f"hc{li}_{ci_}", tag="hc")
                    nc.sync.dma_start(out=hc[:, :ln], in_=h_prev[:ci, off:off + ln])
                    ac = of_pool.tile([32, NODE_CH], F32, name=f"ac{li}_{ci_}", tag="ac")
                    nc.sync.dma_start(out=ac[:dd, :ln], in_=agg_dram[:dd, off:off + ln])
                    ps = node_psp.tile([co, NODE_CH], F32, name=f"nps{li}_{ci_}", tag="nps")
                    nc.tensor.matmul(out=ps[:, :ln], lhsT=wroot[:],
                                     rhs=hc[:, :ln], start=True, stop=False)
                    nc.tensor.matmul(out=ps[:, :ln], lhsT=wrel_ap,
                                     rhs=ac[:dd, :ln], start=False, stop=True)
                    of = of_pool.tile([co, NODE_CH], F32, name=f"of{li}_{ci_}", tag="of")
                    nc.vector.tensor_copy(out=of[:, :ln], in_=ps[:, :ln])
                    real = max(0, min(NSHARD - off, ln))
                    if real:
                        nc.vector.tensor_reduce(out=stats_s[:co, ci_:ci_ + 1],
                                                in_=of[:, :real], axis=mybir.AxisListType.X,
                                                op=ALU.add)
                        sqs = sq_pool.tile([co, NODE_CH], F32, name=f"sqs{li}_{ci_}", tag="sqs")
                        nc.scalar.activation(out=sqs[:, :real], in_=of[:, :real],
                                             func=AF.Square,
                                             accum_out=stats_q[:co, ci_:ci_ + 1])
                    for j in range(ln // 128):
                        blk = (off + j * 128) // 128
                        tp = tr_psp.tile([128, 64], F32, name=f"tp{li}_{ci_}_{j}", tag="trp")
                        nc.tensor.transpose(out=tp[:, :co], in_=of[:, j * 128:(j + 1) * 128],
                                            identity=ident[:co, :co])
                        nc.vector.tensor_copy(out=pre_nm[:, blk, :co], in_=tp[:, :co])
                return stats_s, stats_q, len(chunks)

            def bn_elu(li, co, pre_nm, e_nm, stats_s, stats_q, nchunks):
                """stats allreduce -> affine params -> NM affine+ELU in pre_nm."""
                srow = sm_pool.tile([64, 2], F32, name=f"st{li}", tag=f"st{li}")
                nc.vector.tensor_reduce(out=srow[:co, 0:1], in_=stats_s[:co, :nchunks],
                                        axis=mybir.AxisListType.X, op=ALU.add)
                nc.vector.tensor_reduce(out=srow[:co, 1:2], in_=stats_q[:co, :nchunks],
                                        axis=mybir.AxisListType.X, op=ALU.add)
                arin = dram.tile([co, 2], F32, name=f"arin{li}", tag=f"arin{li}")
                arout = dram.tile([co, 2], F32, name=f"arout{li}", tag=f"arout{li}",
                                  addr_space="Shared")
                nc.sync.dma_start(out=arin[:], in_=srow[:co, :])
                nc.gpsimd.collective_compute(
                    "AllReduce", ALU.add, replica_groups=[list(range(NCORES))],
                    ins=[arin.opt()], outs=[arout.opt()])
                # rows [1, co]
                rows = sm_pool.tile([1, 8 * 64], F32, name=f"rw{li}", tag=f"rw{li}")
                mean_r, q_r = rows[:, 0:co], rows[:, 64:64 + co]
                var_r, a_r = rows[:, 128:128 + co], rows[:, 192:192 + co]
                b_r, t_r = rows[:, 256:256 + co], rows[:, 320:320 + co]
                nc.sync.dma_start(out=mean_r, in_=bass.AP(arout.tensor, arout.offset, [[0, 1], [2, co]]))
                nc.sync.dma_start(out=q_r, in_=bass.AP(arout.tensor, arout.offset + 1, [[0, 1], [2, co]]))
                nc.vector.tensor_scalar_mul(out=mean_r, in0=mean_r, scalar1=1.0 / N)
                nc.vector.tensor_scalar_mul(out=q_r, in0=q_r, scalar1=1.0 / N)
                nc.vector.tensor_tensor(out=var_r, in0=mean_r, in1=mean_r, op=ALU.mult)
                nc.vector.tensor_tensor(out=var_r, in0=q_r, in1=var_r, op=ALU.subtract)
                nc.scalar.activation(out=var_r, in_=var_r, func=AF.Sqrt, bias=EPS)
                nc.vector.reciprocal(out=var_r, in_=var_r)
                gam = cpool_row(li)
                nc.vector.tensor_tensor(out=a_r, in0=var_r, in1=wsb[f"gam{li}"][:], op=ALU.mult)
                nc.vector.tensor_tensor(out=t_r, in0=mean_r, in1=a_r, op=ALU.mult)
                nc.vector.tensor_tensor(out=b_r, in0=wsb[f"bet{li}"][:], in1=t_r, op=ALU.subtract)
                del gam
                abd = dram.tile([2, 64], F32, name=f"abd{li}", tag=f"abd{li}")
                nc.sync.dma_start(out=abd[0:1, :co], in_=a_r)
                nc.sync.dma_start(out=abd[1:2, :co], in_=b_r)
                ab_bc = sm_pool.tile([128, 2, 64], F32, name=f"abbc{li}", tag=f"abbc{li}")
                nc.sync.dma_start(out=ab_bc[:],
                                  in_=bass.AP(abd.tensor, abd.offset, [[0, 128], [64, 2], [1, 64]]))
                # NM affine + ELU (in place in pre_nm)
                av = ab_bc[:, 0, :co]
                bv = ab_bc[:, 1, :co]
                a_b = bass.AP(av.tensor, av.offset, [av.ap[0], [0, NBLK], [1, co]])
                b_b = bass.AP(bv.tensor, bv.offset, [bv.ap[0], [0, NBLK], [1, co]])
                pnm = pre_nm[:, :, :co]
                nc.vector.tensor_tensor(out=pnm, in0=pnm, in1=a_b, op=ALU.mult)
                nc.vector.tensor_tensor(out=pnm, in0=pnm, in1=b_b, op=ALU.add)
                enm = e_nm[:, :, :co]
                nc.vector.tensor_scalar_min(out=enm, in0=pnm, scalar1=0.0)
                nc.scalar.activation(out=enm, in_=enm, func=AF.Exp)
                nc.vector.tensor_scalar_add(out=enm, in0=enm, scalar1=-1.0)
                nc.vector.tensor_tensor(out=pnm, in0=pnm, in1=enm, op=ALU.max)

            def cpool_row(li):
                return None

            # ---------------- layer loop ----------------
            h_prev = x_in[:]
            pre_nm = nm_pool.tile([128, NBLK, 64], F32, name="pre_nm")
            e_nm = nm_pool.tile([128, NBLK, 64], F32, name="e_nm")

            for li, (ci, co, dd, pre_flag) in enumerate(LAYERS):
                # --- table prep
                agin_sb = nm_pool.tile([128, NBLK, dd], BF16, name=f"aginsb{li}", tag="aginsb")
                if li == 0:
                    # xp1 = x @ w_rel1, node-major bf16
                    nch = NPAD // NODE_CH
                    chunks = [(i * NODE_CH, NODE_CH) for i in range(nch)]
                    if NPAD % NODE_CH:
                        chunks.append((nch * NODE_CH, NPAD % NODE_CH))
                    for ci_, (off, ln) in enumerate(chunks):
                        xc = of_pool.tile([64, NODE_CH], F32, name=f"xc{ci_}", tag="hc")
                        nc.sync.dma_start(out=xc[:, :ln], in_=x_in[:, off:off + ln])
                        ps = node_psp.tile([16, NODE_CH], F32, name=f"xps{ci_}", tag="nps")
                        nc.tensor.matmul(out=ps[:, :ln], lhsT=wsb["wrel0"][:],
                                         rhs=xc[:, :ln], start=True, stop=True)
                        of = of_pool.tile([16, NODE_CH], F32, name=f"xof{ci_}", tag="of")
                        nc.vector.tensor_copy(out=of[:, :ln], in_=ps[:, :ln])
                        for j in range(ln // 128):
                            blk = (off + j * 128) // 128
                            tp = tr_psp.tile([128, 64], F32, name=f"xtp{ci_}_{j}", tag="trp")
                            nc.tensor.transpose(out=tp[:, :16], in_=of[:, j * 128:(j + 1) * 128],
                                                identity=ident[:16, :16])
                            nc.vector.tensor_copy(out=agin_sb[:, blk, :], in_=tp[:, :16])
                else:
                    nc.vector.tensor_copy(out=agin_sb[:], in_=pre_nm[:, :, :dd])
                table_prep(li, agin_sb, dd)

                # --- gather + aggregate
                gather_aggregate(li, dd)

                # --- dense node compute
                stats_s, stats_q, nchunks = node_compute(li, ci, co, dd, pre_flag,
                                                         h_prev, pre_nm, e_nm)
                bn_elu(li, co, pre_nm, e_nm, stats_s, stats_q, nchunks)

                # h_fm via transposes back -> DRAM (skip for last layer: pooling
                # consumes the node-major form directly)
                if li < len(LAYERS) - 1:
                    for ch4 in range((NBLK + 3) // 4):
                        blks = range(ch4 * 4, min((ch4 + 1) * 4, NBLK))
                        tb_sb = of_pool.tile([64, NODE_CH], F32, name=f"tb{li}_{ch4}", tag="tbsb")
                        for j, blk in enumerate(blks):
                            tp2 = tr2_psp.tile([64, 128], F32, name=f"tb{li}_{blk}", tag="tr2")
                            nc.tensor.transpose(out=tp2[:co, :], in_=pre_nm[:, blk, :co],
                                                identity=ident[:128, :128])
                            nc.vector.tensor_copy(out=tb_sb[:co, j * 128:(j + 1) * 128],
                                                  in_=tp2[:co, :])
                        ln4 = len(blks) * 128
                        nc.sync.dma_start(
                            out=h_dram[li][:co, ch4 * NODE_CH:ch4 * NODE_CH + ln4],
                            in_=tb_sb[:co, :ln4])
                    h_prev = h_dram[li][:]

            # ---------------- pooling + MLP + log_softmax ----------------
            pool_ps = [node_psp.tile([128, 64], F32, name=f"pps{h}", tag="nps") for h in range(2)]
            for blk in range(NBLK):
                ghc = gh_pool.tile([128, NUM_GRAPHS], BF16, name=f"ghc{blk}", tag="ghc")
                # graph one-hot for this 128-node block: (bid == iota256)
                bb = bid_sb[:, blk:blk + 1]
                bid_bc = bass.AP(bb.tensor, bb.offset, [bb.ap[0], [0, NUM_GRAPHS]])
                nc.vector.tensor_tensor(out=ghc[:], in0=bid_bc, in1=iota256[:],
                                        op=ALU.is_equal)
                hbf = gh_pool.tile([128, 64], BF16, name=f"hbf{blk}", tag="hbf")
                nc.vector.tensor_copy(out=hbf[:], in_=pre_nm[:, blk, :64])
                for h in range(2):
                    nc.tensor.matmul(out=pool_ps[h][:], lhsT=ghc[:, h * 128:(h + 1) * 128],
                                     rhs=hbf[:], start=(blk == 0), stop=(blk == NBLK - 1))
            pooled_nm = sm_pool.tile([128, 2, 64], F32, name="pooled_nm")
            for h in range(2):
                nc.vector.tensor_copy(out=pooled_nm[:, h, :], in_=pool_ps[h][:])
            par_in = dram.tile([NUM_GRAPHS, 64], F32, name="par_in")
            par_out = dram.tile([NUM_GRAPHS, 64], F32, name="par_out", addr_space="Shared")
            par_ap = bass.AP(par_in.tensor, par_in.offset, [[64, 128], [128 * 64, 2], [1, 64]])
            nc.sync.dma_start(out=par_ap, in_=pooled_nm[:])
            nc.gpsimd.collective_compute(
                "AllReduce", ALU.add, replica_groups=[list(range(NCORES))],
                ins=[par_in.opt()], outs=[par_out.opt()])
            nc.sync.dma_start(out=pooled_nm[:],
                              in_=bass.AP(par_out.tensor, par_out.offset,
                                          [[64, 128], [128 * 64, 2], [1, 64]]))
            pooled_fm = sm_pool.tile([64, NUM_GRAPHS], F32, name="pooled_fm")
            for h in range(2):
                nc.scalar.activation(out=pooled_nm[:, h, :], in_=pooled_nm[:, h, :],
                                     func=AF.Copy, scale=grecip_sb[:, h:h + 1])
                tp2 = tr2_psp.tile([64, 128], F32, name=f"ptr{h}", tag="tr2")
                nc.tensor.transpose(out=tp2[:], in_=pooled_nm[:, h, :],
                                    identity=ident[:])
                nc.vector.tensor_copy(out=pooled_fm[:, h * 128:(h + 1) * 128], in_=tp2[:])
            z_ps = node_psp.tile([64, NUM_GRAPHS], F32, name="z_ps", tag="nps")
            nc.tensor.matmul(out=z_ps[:], lhsT=wsb["wlin1"][:], rhs=pooled_fm[:],
                             start=True, stop=True)
            z_fm = sm_pool.tile([64, NUM_GRAPHS], F32, name="z_fm")
            nc.scalar.activation(out=z_fm[:], in_=z_ps[:], func=AF.Relu,
                                 bias=wsb["blin1"][:])
            lg_ps = node_psp.tile([10, NUM_GRAPHS], F32, name="lg_ps", tag="nps")
            nc.tensor.matmul(out=lg_ps[:], lhsT=wsb["wlin2"][:], rhs=z_fm[:],
                             start=True, stop=True)
            logits_fm = sm_pool.tile([10, NUM_GRAPHS], F32, name="logits_fm")
            nc.scalar.activation(out=logits_fm[:], in_=lg_ps[:], func=AF.Identity,
                                 bias=wsb["blin2"][:])
            lnm = sm_pool.tile([128, 2, 10], F32, name="lnm")
            mrow = sm_pool.tile([128, 4], F32, name="mrow")
            for h in range(2):
                tp3 = tr_psp.tile([128, 64], F32, name=f"ltr{h}", tag="trp")
                nc.tensor.transpose(out=tp3[:, :10], in_=logits_fm[:, h * 128:(h + 1) * 128],
                                    identity=ident[:10, :10])
                nc.vector.tensor_copy(out=lnm[:, h, :], in_=tp3[:, :10])
                nc.vector.tensor_reduce(out=mrow[:, h:h + 1], in_=lnm[:, h, :],
                                        axis=mybir.AxisListType.X, op=ALU.max)
                nc.vector.tensor_scalar(out=lnm[:, h, :], in0=lnm[:, h, :],
                                        scalar1=mrow[:, h:h + 1], scalar2=None,
                                        op0=ALU.subtract)
                esb = sm_pool.tile([128, 10], F32, name=f"esb{h}", tag="esb")
                nc.scalar.activation(out=esb[:], in_=lnm[:, h, :], func=AF.Exp)
                nc.vector.tensor_reduce(out=mrow[:, 2 + h:3 + h], in_=esb[:],
                                        axis=mybir.AxisListType.X, op=ALU.add)
                nc.scalar.activation(out=mrow[:, 2 + h:3 + h], in_=mrow[:, 2 + h:3 + h],
                                     func=AF.Ln)
                nc.vector.tensor_scalar(out=lnm[:, h, :], in0=lnm[:, h, :],
                                        scalar1=mrow[:, 2 + h:3 + h], scalar2=None,
                                        op0=ALU.subtract)
                nc.sync.dma_start(out=out_o[h * 128:(h + 1) * 128, :], in_=lnm[:, h, :])

    nc.compile()
    return nc


def kernel(**inputs):
    x = inputs["x"]
    edge_index = inputs["edge_index"]
    batch = inputs["batch"]
    meta, percore, shared = _preprocess(x, edge_index, batch)

    nc = _build(meta, inputs)

    ident = np.eye(128, dtype=np.float32)
    iota = np.tile(np.arange(W, dtype=np.float32), (128, 1)).astype(ml_dtypes.bfloat16)
    iota256 = np.tile(np.arange(NUM_GRAPHS, dtype=np.float32), (128, 1)).astype(ml_dtypes.bfloat16)
    in_maps = []
    for k in range(NCORES):
        m = {
            "x_fm": percore["x_fm"][k],
            "idx": percore["idx"][k],
            "dstw": percore["dstw"][k],
            "recip_1r": percore["recip_1r"][k],
            "bid": percore["bid"][k],
            "iota256": iota256,
            "grecip": shared["grecip"],
            "ident": ident,
            "iota": iota,
            "wlin1": np.asarray(inputs["w_lin1"], np.float32),
            "blin1": np.asarray(inputs["b_lin1"], np.float32).reshape(64, 1),
            "wlin2": np.asarray(inputs["w_lin2"], np.float32),
            "blin2": np.asarray(inputs["b_lin2"], np.float32).reshape(10, 1),
        }
        for li in range(4):
            m[f"wroot{li}"] = np.asarray(inputs[f"w_root{li + 1}"], np.float32)
            m[f"wrel{li}"] = np.asarray(inputs[f"w_rel{li + 1}"], np.float32)
            m[f"gam{li}"] = np.asarray(inputs[f"g{li + 1}"], np.float32).reshape(1, -1)
            m[f"bet{li}"] = np.asarray(inputs[f"be{li + 1}"], np.float32).reshape(1, -1)
        in_maps.append(m)

    global _LAST
    _LAST = (nc, in_maps)
    res = run_bass_kernel_spmd(nc, in_maps, list(range(NCORES)))
    return np.asarray(res.results[0]["out"], dtype=np.float32)


_LAST = None


def rerun():
    """Re-execute the last compiled kernel (for timing)."""
    import time
    nc, in_maps = _LAST
    t0 = time.time()
    run_bass_kernel_spmd(nc, in_maps, list(range(NCORES)))
    return time.time() - t0


if __name__ == "__main__":
    import reference
    ins = {k: np.asarray(v) for k, v in reference.setup_inputs().items()}
    out = kernel(**ins)
    print("kernel out", out.shape, out.dtype, out[:2])

